# revision 13
# baseline (speedup 1.0000x reference)
"""Trainium2 Bass kernel for the 4-branch "Memory multimode" attention module.

Reference computation (per batch element b):
    q  = q_key[b].reshape(1024, 128)        (row-major reinterpret)
    pq = p_q_key[b].reshape(1024, 128)
    k  = m_key[b].reshape(128, 2048)
    pk = p_m_key[b].reshape(128, 2048)
    mval = m_val[b].reshape(512, 2048).T    # [2048, 512]
    out  = (sm(q@k) + sm(pq@pk) + sm(pq@k) + sm(q@pk)) @ mval
    where sm() is softmax over the QUERY dim (axis 0 of each [1024, 2048] score
    matrix).  Final output channel-concats q_val.

Key algebraic point: all four branches share the same value matrix, so the four
softmax matrices are summed BEFORE the value matmul - one [1024,2048]@[2048,512]
matmul instead of four (2.5x FLOP reduction vs the naive form).

Implementation (one NeuronCore per batch element, 8 cores, data-parallel):
  * Transposed score layout S^T = [key_pos(l) x query(i)]: softmax reduction
    runs along the free dim; S^T tiles come straight off the PE with
    lhsT = keys l-tile (natural layout) and rhs = Q^T (host pre-transposed).
  * Single-pass fp16 score matmuls (1 cyc/row on the PE, ~11-bit operand
    mantissa).  The correctness budget (rel err < 2e-2 vs max|out|) has >10x
    headroom over the ~2e-3 this costs end-to-end.
  * No max-subtraction needed: |scores| <= ~75, exp stays in fp32/bf16
    exponent range.  ScalarE exp emits bf16 E tiles (bf16 keeps the fp32
    exponent, so no overflow) plus fused row-sum denominators via accum_out.
    The exp sweep over 8M scores is the bottleneck engine (~79us busy);
    everything else is arranged to hide under it.
  * The 1/D scaling + 4-branch sum runs on the VectorE as tensor_scalar (4x
    mode) + tensor_tensor (2x) ops - scalar_tensor_tensor has no fast DVE
    uops - writing A^T to SBUF as fp16 for the value matmul.
  * Value matmul in fp16 (1 cyc/row); n_overlap of its 8 output-row PSUM
    accumulations interleave into phase 1, one matmul after each score
    branch, lagged two l-tiles so the (strict-FIFO) PE never stalls on the
    DVE chain and stays HAM-warm.
  * SBUF/PSUM tiles are consolidated into a handful of big tiles with
    hand-rotated slices: every allocated tile costs a semaphore, and every
    engine queue replays ~150ns per semaphore init before its first real
    instruction - tile count sets the kernel ramp.
  * Output staged to SBUF as fp16 by the DVE (ScalarE stays exp-only until
    the end); host upcasts and concatenates q_val.
"""

import numpy as np

import concourse.bass as bass
import concourse.mybir as mybir
import concourse.tile as tile
from concourse.bass_utils import run_bass_kernel_spmd
from concourse.vector_clock import ScopedClock

# The walrus build in this image supports only ONE sync-wait command per
# instruction (CTRL_NO_STRUCT / S3_LW_STRUCT encodings); this concourse's Tile
# scheduler freely attaches several.  Two fixes: (1) split the kernel-tail
# drain's waits over several drains, (2) a post-scheduling pass that moves
# overflow waits onto NoOps inserted before the over-subscribed instruction.
_MAX_WAITS = 1


def _split_drain_and_barrier(self, tick_clock, wait_clock):
    nc = self.nc
    drain_inst = nc.sync.drain()
    wait_clock.add_sem_waits(
        drain_inst.ins, ScopedClock({None: tick_clock.global_clock})
    )
    mi = drain_inst.ins
    waits = list(mi.sync_info.on_wait)
    if len(waits) > _MAX_WAITS:
        del mi.sync_info.on_wait[_MAX_WAITS:]
        rest = waits[_MAX_WAITS:]
        for i in range(0, len(rest), _MAX_WAITS):
            extra = nc.sync.drain()
            if extra.ins.sync_info is None:
                extra.ins.sync_info = mybir.SyncInfo(on_wait=[], on_update=[])
            extra.ins.sync_info.on_wait.extend(rest[i : i + _MAX_WAITS])

    nc.all_engine_barrier()
    assert self.sems is not None
    popped = nc._tile_sem_poison_stack.pop()
    assert popped is self._sem_poison
    nc.clear_and_free_semaphores(list(self.sems.allocated().values()))
    nc.all_engine_barrier()


tile.TileContext._drain_and_barrier = _split_drain_and_barrier


def _split_sync_waits(nc, cap: int = _MAX_WAITS):
    for f in nc.m.functions:
        for blk in f.blocks:
            out = []
            changed = False
            for inst in blk.instructions:
                si = inst.sync_info
                if si is not None and len(si.on_wait) > cap:
                    waits = list(si.on_wait)
                    rest, keep = waits[:-cap], waits[-cap:]
                    for i in range(0, len(rest), cap):
                        noop = mybir.InstNoOp(
                            name=nc.get_next_instruction_name(), ins=[], outs=[]
                        )
                        noop.engine = inst.engine
                        noop.sync_info = mybir.SyncInfo(
                            on_wait=rest[i : i + cap], on_update=[]
                        )
                        nc.register_instruction(noop)
                        out.append(noop)
                    inst.sync_info = mybir.SyncInfo(
                        on_wait=keep, on_update=list(si.on_update)
                    )
                    changed = True
                out.append(inst)
            if changed:
                blk.instructions = out
    return nc


B, H, W = 8, 32, 32
HW = H * W          # 1024 queries
KD = 128            # key dim
VD = 512            # val dim
L = 2 * HW          # 2048 key positions per key matrix
NT = L // 128       # 16 l-tiles
NO = HW // 128      # 8 output row-tiles
NCORES = 8

F32 = mybir.dt.float32
BF16 = mybir.dt.bfloat16
F16 = mybir.dt.float16

_nc_cache = {}


def build_nc(n_overlap: int = 4, n_warm: int = 12):
    nc = bass.Bass("TRN2", target_bir_lowering=False, debug=False)

    def din(name, shape, dt):
        return nc.dram_tensor(name, shape, dt, kind="ExternalInput").ap()

    kt_d = din("kt", [KD, 2 * L], F16)    # m_key | p_m_key, fp16
    qt_d = din("qt", [KD, 2 * HW], F16)   # q^T | pq^T, fp16
    mvt_d = din("mvt", [L, VD], F16)      # m_val reinterpreted+transposed, fp16
    out_d = nc.dram_tensor("out", [HW, VD], F16, kind="ExternalOutput").ap()

    EXP = mybir.ActivationFunctionType.Exp

    with tile.TileContext(nc) as tc:
        with (
            tc.tile_pool(name="sb", bufs=1) as sb,
            tc.tile_pool(name="ps", bufs=1, space="PSUM") as ps,
        ):
            qt = sb.tile([128, 2 * HW], F16, tag="qt")
            kt = sb.tile([128, 2 * L], F16, tag="kt")
            mv = sb.tile([128, NT * VD], F16, tag="mv")
            # E tiles: (t%2, br) slices; bf16 keeps the fp32 exponent range
            et = sb.tile([128, 8 * HW], BF16, tag="et")
            at = sb.tile([128, NT * HW], F16, tag="at")   # A^T, all 16 tiles
            ut = sb.tile([128, 6 * HW], F16, tag="ut")    # DVE chain scratch
            dt_ = sb.tile([128, 8], F32, tag="dt")        # denoms, (t%2)*4+br
            it_ = sb.tile([128, 4], F32, tag="it")        # 1/denoms
            ot = sb.tile([128, 2 * VD], F16, tag="ot")    # out staging, i%2
            warm = sb.tile([128, 512], F16, tag="warm")
            # PSUM: score tiles (br%2 -> 2 banks each) + 4 output-row banks
            s_ps = ps.tile([128, 2 * HW], F32, tag="s_ps")
            o_ps = ps.tile([128, n_overlap * VD], F32, tag="o_ps")

            # PE warm-up: the HAM clock gate keeps an idle PE at 1.2 GHz and
            # takes ~3.4us of sustained activity to release.  Dummy matmuls
            # on a memset scratch tile span the DMA wait so the real matmuls
            # start at 2.4 GHz; results land in o_ps[:, 0:512] and are
            # discarded by its first start=True accumulation.
            nc.gpsimd.memset(warm[:], 0)
            for w in range(n_warm):
                nc.tensor.matmul(o_ps[:, 0:VD], warm[:, 0:128], warm[:],
                                 start=True, stop=True)

            # ---- input loads.  All issued from the GpSimd sequencer (the
            # emptiest queue): every queue replays the semaphore-init
            # preamble, and the Sync queue is busiest.  First-needed chunks
            # are 64KB so the first score matmul starts after ~3us of DMA.
            for c in range(4):
                nc.gpsimd.dma_start(qt[:, c * 256 : (c + 1) * 256],
                                    qt_d[:, c * 256 : (c + 1) * 256])
            for c in range(2):
                nc.gpsimd.dma_start(kt[:, c * 256 : (c + 1) * 256],
                                    kt_d[:, c * 256 : (c + 1) * 256])
            for c in range(2):
                nc.gpsimd.dma_start(qt[:, 1024 + c * 512 : 1024 + (c + 1) * 512],
                                    qt_d[:, 1024 + c * 512 : 1024 + (c + 1) * 512])
            nc.gpsimd.dma_start(kt[:, L : L + 512], kt_d[:, L : L + 512])
            for q in range(1, 4):  # remaining key quarters, y-interleaved
                for y in range(2):
                    sl = slice(y * L + q * 512, y * L + (q + 1) * 512)
                    nc.gpsimd.dma_start(kt[:, sl], kt_d[:, sl])
            for c in range(2):
                nc.gpsimd.dma_start(mv[:, c * 256 : (c + 1) * 256],
                                    mvt_d[0:128, c * 256 : (c + 1) * 256])
            for t in range(1, NT):
                nc.sync.dma_start(
                    mv[:, t * VD : (t + 1) * VD], mvt_d[t * 128 : (t + 1) * 128, :]
                )

            MUL = mybir.AluOpType.mult

            def value_mm(t, i):
                nc.tensor.matmul(
                    o_ps[:, i * VD : (i + 1) * VD],
                    at[:, t * HW + i * 128 : t * HW + (i + 1) * 128],
                    mv[:, t * VD : (t + 1) * VD],
                    start=(t == 0),
                    stop=(t == NT - 1),
                )

            # ---- phase 1 ---------------------------------------------------
            for t in range(NT):
                e_sl = []
                for y in range(2):
                    for xh in range(2):
                        br = 2 * y + xh
                        s_sl = s_ps[:, (br % 2) * HW : (br % 2 + 1) * HW]
                        for c in range(2):
                            nc.tensor.matmul(
                                s_sl[:, c * 512 : (c + 1) * 512],
                                kt[:, y * L + t * 128 : y * L + (t + 1) * 128],
                                qt[:, xh * HW + c * 512 : xh * HW + (c + 1) * 512],
                                start=True, stop=True)
                        # E^T = exp(S^T) in bf16; accum_out = row sum = denom
                        e_t = et[:, ((t % 2) * 4 + br) * HW :
                                 ((t % 2) * 4 + br + 1) * HW]
                        nc.scalar.activation(
                            e_t, s_sl, EXP,
                            accum_out=dt_[:, (t % 2) * 4 + br :
                                          (t % 2) * 4 + br + 1],
                        )
                        e_sl.append(e_t)
                        # one value matmul per branch, two l-tiles behind:
                        # keeps the PE streaming (HAM warm) with zero stalls
                        if t >= 2:
                            value_mm(t - 2, br)

                dcols = dt_[:, (t % 2) * 4 : (t % 2) * 4 + 4]
                if t == NT - 1:
                    # last tile: reciprocal + first 3 scales run while the
                    # 4th EXP still streams, shortening the kernel tail
                    nc.vector.reciprocal(it_[:, 0:3], dcols[:, 0:3])
                else:
                    nc.vector.reciprocal(it_[:], dcols)

                # A^T[t] = sum_br invD_br * E_br
                a_sl = at[:, t * HW : (t + 1) * HW]
                u = [ut[:, j * HW : (j + 1) * HW] for j in range(6)]
                for j in range(3):
                    nc.vector.tensor_scalar_mul(u[j], e_sl[j], it_[:, j : j + 1])
                nc.vector.tensor_add(u[4], u[0], u[1])
                if t == NT - 1:
                    nc.vector.reciprocal(it_[:, 3:4], dcols[:, 3:4])
                nc.vector.tensor_scalar_mul(u[3], e_sl[3], it_[:, 3:4])
                nc.vector.tensor_add(u[5], u[2], u[3])
                nc.vector.tensor_add(a_sl, u[4], u[5])

            for t in (NT - 2, NT - 1):
                for i in range(n_overlap):
                    value_mm(t, i)

            # ---- phase 2: drain overlapped rows, then the remaining rows ---
            for i in range(NO):
                if i < n_overlap:
                    o_row = o_ps[:, i * VD : (i + 1) * VD]
                else:
                    o_row = s_ps[:, (i % 2) * HW : (i % 2) * HW + VD]
                    for t in range(NT):
                        nc.tensor.matmul(
                            o_row,
                            at[:, t * HW + i * 128 : t * HW + (i + 1) * 128],
                            mv[:, t * VD : (t + 1) * VD],
                            start=(t == 0),
                            stop=(t == NT - 1),
                        )
                o_sb = ot[:, (i % 2) * VD : (i % 2 + 1) * VD]
                # DVE stages (and downcasts) the output; each row goes out as
                # 2 DMAs so the last row drains on 2 rings.
                nc.vector.tensor_copy(o_sb, o_row)
                for c in range(2):
                    nc.scalar.dma_start(
                        out_d[i * 128 : (i + 1) * 128, c * 256 : (c + 1) * 256],
                        o_sb[:, c * 256 : (c + 1) * 256])

    _split_sync_waits(nc)
    return nc


def make_in_maps(m_key, m_val, q_key, p_m_key, p_q_key):
    in_maps = []
    for b in range(B):
        kt = np.empty((KD, 2 * L), np.float16)
        kt[:, :L] = m_key[b].reshape(KD, L)
        kt[:, L:] = p_m_key[b].reshape(KD, L)
        qt = np.empty((KD, 2 * HW), np.float16)
        qt[:, :HW] = q_key[b].reshape(HW, KD).T
        qt[:, HW:] = p_q_key[b].reshape(HW, KD).T
        mvt = np.ascontiguousarray(
            m_val[b].reshape(VD, L).T.astype(np.float16))
        in_maps.append({"kt": kt, "qt": qt, "mvt": mvt})
    return in_maps


def run(inputs, trace: bool = False, n_overlap: int = 4, n_warm: int = 12):
    """Run on the 8 NeuronCores; returns (full_output, BassKernelResults)."""
    inputs = {k: np.asarray(v, dtype=np.float32) for k, v in inputs.items()}
    key = (n_overlap, n_warm)
    if key not in _nc_cache:
        _nc_cache[key] = build_nc(n_overlap, n_warm)
    nc = _nc_cache[key]
    in_maps = make_in_maps(
        inputs["m_key"], inputs["m_val"], inputs["q_key"],
        inputs["p_m_key"], inputs["p_q_key"],
    )
    res = run_bass_kernel_spmd(nc, in_maps, list(range(NCORES)), trace=trace)
    q_val = inputs["q_val"]
    outs = []
    for b in range(B):
        mat = np.asarray(res.results[b]["out"]).astype(np.float32)
        attn = mat.reshape(VD, H, W)                 # reinterpret, no transpose
        outs.append(np.concatenate([attn, q_val[b]], axis=0))
    return np.stack(outs), res


def kernel(**inputs) -> np.ndarray:
    out, _ = run(inputs, trace=False)
    return out


# revision 15
# speedup vs baseline: 1.5672x; 1.5672x over previous
"""Trainium2 Bass kernel for the 4-branch "Memory multimode" attention module.

Reference computation (per batch element b):
    q  = q_key[b].reshape(1024, 128)        (row-major reinterpret)
    pq = p_q_key[b].reshape(1024, 128)
    k  = m_key[b].reshape(128, 2048)
    pk = p_m_key[b].reshape(128, 2048)
    mval = m_val[b].reshape(512, 2048).T    # [2048, 512]
    out  = (sm(q@k) + sm(pq@pk) + sm(pq@k) + sm(q@pk)) @ mval
    where sm() is softmax over the QUERY dim (axis 0 of each [1024, 2048] score
    matrix).  Final output channel-concats q_val.

Key algebraic point: all four branches share the same value matrix, so the four
softmax matrices are summed BEFORE the value matmul - one [1024,2048]@[2048,512]
matmul instead of four (2.5x FLOP reduction vs the naive form).

Implementation (one NeuronCore per batch element, 8 cores, data-parallel):
  * Transposed score layout S^T = [key_pos(l) x query(i)]: softmax reduction
    runs along the free dim; S^T tiles come straight off the PE with
    lhsT = keys l-tile (natural layout) and rhs = Q^T (host pre-transposed).
  * Single-pass fp16 score matmuls (1 cyc/row on the PE, ~11-bit operand
    mantissa).  The correctness budget (rel err < 2e-2 vs max|out|) has >10x
    headroom over the ~2e-3 this costs end-to-end.
  * No max-subtraction needed: |scores| <= ~75, exp stays in fp32/bf16
    exponent range.  ScalarE exp emits bf16 E tiles (bf16 keeps the fp32
    exponent, so no overflow) plus fused row-sum denominators via accum_out.
    The exp sweep over 8M scores is the bottleneck engine (~79us busy);
    everything else is arranged to hide under it.
  * The 1/D scaling + 4-branch sum runs on the VectorE as tensor_scalar (4x
    mode) + tensor_tensor (2x) ops - scalar_tensor_tensor has no fast DVE
    uops - writing A^T to SBUF as fp16 for the value matmul.
  * Value matmul in fp16 (1 cyc/row); n_overlap of its 8 output-row PSUM
    accumulations interleave into phase 1, one matmul after each score
    branch, lagged two l-tiles so the (strict-FIFO) PE never stalls on the
    DVE chain and stays HAM-warm.
  * SBUF/PSUM tiles are consolidated into a handful of big tiles with
    hand-rotated slices: every allocated tile costs a semaphore, and every
    engine queue replays ~150ns per semaphore init before its first real
    instruction - tile count sets the kernel ramp.
  * Output staged to SBUF as fp16 by the DVE (ScalarE stays exp-only until
    the end); host upcasts and concatenates q_val.
"""

import numpy as np

import concourse.bass as bass
import concourse.mybir as mybir
import concourse.tile as tile
from concourse.bass_utils import run_bass_kernel_spmd
from concourse.vector_clock import ScopedClock

# The walrus build in this image supports only ONE sync-wait command per
# instruction (CTRL_NO_STRUCT / S3_LW_STRUCT encodings); this concourse's Tile
# scheduler freely attaches several.  Two fixes: (1) split the kernel-tail
# drain's waits over several drains, (2) a post-scheduling pass that moves
# overflow waits onto NoOps inserted before the over-subscribed instruction.
_MAX_WAITS = 1


def _split_drain_and_barrier(self, tick_clock, wait_clock):
    nc = self.nc
    drain_inst = nc.sync.drain()
    wait_clock.add_sem_waits(
        drain_inst.ins, ScopedClock({None: tick_clock.global_clock})
    )
    mi = drain_inst.ins
    waits = list(mi.sync_info.on_wait)
    if len(waits) > _MAX_WAITS:
        del mi.sync_info.on_wait[_MAX_WAITS:]
        rest = waits[_MAX_WAITS:]
        for i in range(0, len(rest), _MAX_WAITS):
            extra = nc.sync.drain()
            if extra.ins.sync_info is None:
                extra.ins.sync_info = mybir.SyncInfo(on_wait=[], on_update=[])
            extra.ins.sync_info.on_wait.extend(rest[i : i + _MAX_WAITS])

    nc.all_engine_barrier()
    assert self.sems is not None
    popped = nc._tile_sem_poison_stack.pop()
    assert popped is self._sem_poison
    nc.clear_and_free_semaphores(list(self.sems.allocated().values()))
    nc.all_engine_barrier()


tile.TileContext._drain_and_barrier = _split_drain_and_barrier


def _split_sync_waits(nc, cap: int = _MAX_WAITS):
    for f in nc.m.functions:
        for blk in f.blocks:
            out = []
            changed = False
            for inst in blk.instructions:
                si = inst.sync_info
                if si is not None and len(si.on_wait) > cap:
                    waits = list(si.on_wait)
                    rest, keep = waits[:-cap], waits[-cap:]
                    for i in range(0, len(rest), cap):
                        noop = mybir.InstNoOp(
                            name=nc.get_next_instruction_name(), ins=[], outs=[]
                        )
                        noop.engine = inst.engine
                        noop.sync_info = mybir.SyncInfo(
                            on_wait=rest[i : i + cap], on_update=[]
                        )
                        nc.register_instruction(noop)
                        out.append(noop)
                    inst.sync_info = mybir.SyncInfo(
                        on_wait=keep, on_update=list(si.on_update)
                    )
                    changed = True
                out.append(inst)
            if changed:
                blk.instructions = out
    return nc


B, H, W = 8, 32, 32
HW = H * W          # 1024 queries
KD = 128            # key dim
VD = 512            # val dim
L = 2 * HW          # 2048 key positions per key matrix
NT = L // 128       # 16 l-tiles
NO = HW // 128      # 8 output row-tiles
NCORES = 8

F32 = mybir.dt.float32
BF16 = mybir.dt.bfloat16
F16 = mybir.dt.float16

_nc_cache = {}


def build_nc(n_overlap: int = 4, n_warm: int = 12):
    nc = bass.Bass("TRN2", target_bir_lowering=False, debug=False)

    def din(name, shape, dt):
        return nc.dram_tensor(name, shape, dt, kind="ExternalInput").ap()

    kt_d = din("kt", [KD, 2 * L], F16)    # m_key | p_m_key, fp16
    qt_d = din("qt", [KD, 2 * HW], F16)   # q^T | pq^T, fp16
    mvt_d = din("mvt", [L, VD], F16)      # m_val reinterpreted+transposed, fp16
    out_d = nc.dram_tensor("out", [HW, VD], F16, kind="ExternalOutput").ap()

    EXP = mybir.ActivationFunctionType.Exp

    with tile.TileContext(nc) as tc:
        with (
            tc.tile_pool(name="keys", bufs=1) as keys_pool,
            tc.tile_pool(name="qts", bufs=1) as qt_pool,
            tc.tile_pool(name="mv", bufs=1) as mv_pool,
            tc.tile_pool(name="ework", bufs=2) as e_pool,
            tc.tile_pool(name="atiles", bufs=1) as a_pool,
            tc.tile_pool(name="dwork", bufs=2) as d_pool,
            tc.tile_pool(name="ostage", bufs=2) as out_pool,
            tc.tile_pool(name="psum_s", bufs=2, space="PSUM") as psum_s,
            tc.tile_pool(name="psum_o", bufs=1, space="PSUM") as psum_o,
        ):
            qt = qt_pool.tile([128, 2 * HW], F16, tag="qt")
            kt = keys_pool.tile([128, 2 * L], F16, tag="kt")
            mv = mv_pool.tile([128, NT * VD], F16, tag="mv")
            warm = d_pool.tile([128, 512], F16, tag="warm", name="warm")
            o_acc = [
                psum_o.tile([128, VD], F32, tag=f"O{i}", name=f"o_acc{i}")
                for i in range(n_overlap)
            ]

            # PE warm-up: the HAM clock gate keeps an idle PE at 1.2 GHz and
            # takes ~3.4us of sustained activity to release.  Dummy matmuls
            # on a memset scratch tile span the DMA wait so the real matmuls
            # start at 2.4 GHz; results land in o_acc[0] and are discarded
            # by its first start=True accumulation.
            nc.gpsimd.memset(warm[:], 0)
            for w in range(n_warm):
                nc.tensor.matmul(o_acc[0][:], warm[:, 0:128], warm[:],
                                 start=True, stop=True)

            # ---- input loads.  All issued from the GpSimd sequencer (the
            # emptiest queue): every queue replays the semaphore-init
            # preamble, and the Sync queue is busiest.  First-needed chunks
            # are 64KB so the first score matmul starts after ~3us of DMA.
            for c in range(4):
                nc.gpsimd.dma_start(qt[:, c * 256 : (c + 1) * 256],
                                    qt_d[:, c * 256 : (c + 1) * 256])
            for c in range(2):
                nc.gpsimd.dma_start(kt[:, c * 256 : (c + 1) * 256],
                                    kt_d[:, c * 256 : (c + 1) * 256])
            for c in range(2):
                nc.gpsimd.dma_start(qt[:, 1024 + c * 512 : 1024 + (c + 1) * 512],
                                    qt_d[:, 1024 + c * 512 : 1024 + (c + 1) * 512])
            nc.gpsimd.dma_start(kt[:, L : L + 512], kt_d[:, L : L + 512])
            for q in range(1, 4):  # remaining key quarters, y-interleaved
                for y in range(2):
                    sl = slice(y * L + q * 512, y * L + (q + 1) * 512)
                    nc.gpsimd.dma_start(kt[:, sl], kt_d[:, sl])
            for c in range(2):
                nc.gpsimd.dma_start(mv[:, c * 256 : (c + 1) * 256],
                                    mvt_d[0:128, c * 256 : (c + 1) * 256])
            for t in range(1, NT):
                nc.sync.dma_start(
                    mv[:, t * VD : (t + 1) * VD], mvt_d[t * 128 : (t + 1) * 128, :]
                )

            a_tiles = []

            def value_mm(t, i):
                nc.tensor.matmul(
                    o_acc[i][:],
                    a_tiles[t][:, i * 128 : (i + 1) * 128],
                    mv[:, t * VD : (t + 1) * VD],
                    start=(t == 0),
                    stop=(t == NT - 1),
                )

            # ---- phase 1 ---------------------------------------------------
            for t in range(NT):
                dtile = d_pool.tile([128, 4], F32, tag="D")
                e_tiles = []
                for y in range(2):
                    for xh in range(2):
                        br = 2 * y + xh
                        s_ps = psum_s.tile([128, HW], F32, tag="S")
                        for c in range(2):
                            nc.tensor.matmul(
                                s_ps[:, c * 512 : (c + 1) * 512],
                                kt[:, y * L + t * 128 : y * L + (t + 1) * 128],
                                qt[:, xh * HW + c * 512 : xh * HW + (c + 1) * 512],
                                start=True, stop=True)
                        # E^T = exp(S^T) in bf16; accum_out = row sum = denom
                        e_t = e_pool.tile([128, HW], BF16, tag=f"E{br}")
                        nc.scalar.activation(
                            e_t[:], s_ps[:], EXP,
                            accum_out=dtile[:, br : br + 1],
                        )
                        e_tiles.append(e_t)
                        # one value matmul per branch, two l-tiles behind:
                        # keeps the PE streaming (HAM warm) with zero stalls
                        if t >= 2:
                            value_mm(t - 2, br)

                invd = d_pool.tile([128, 4], F32, tag="invD")
                if t == NT - 1:
                    # last tile: reciprocal + first 3 scales run while the
                    # 4th EXP still streams, shortening the kernel tail
                    nc.vector.reciprocal(invd[:, 0:3], dtile[:, 0:3])
                else:
                    nc.vector.reciprocal(invd[:], dtile[:])

                # A^T[t] = sum_br invD_br * E_br.  scalar_tensor_tensor has
                # no fast DVE uops (always 1x); tensor_scalar (4x) +
                # tensor_tensor (2x for 16-bit) is ~40% faster.
                a_sb = a_pool.tile([128, HW], F16, tag=f"A{t}")
                u = [d_pool.tile([128, HW], F16, tag=f"u{j}", name=f"u{j}_{t}")
                     for j in range(6)]
                for j in range(3):
                    nc.vector.tensor_scalar_mul(
                        u[j][:], e_tiles[j][:], invd[:, j : j + 1])
                nc.vector.tensor_add(u[4][:], u[0][:], u[1][:])
                if t == NT - 1:
                    nc.vector.reciprocal(invd[:, 3:4], dtile[:, 3:4])
                nc.vector.tensor_scalar_mul(u[3][:], e_tiles[3][:], invd[:, 3:4])
                nc.vector.tensor_add(u[5][:], u[2][:], u[3][:])
                nc.vector.tensor_add(a_sb[:], u[4][:], u[5][:])
                a_tiles.append(a_sb)

            for t in (NT - 2, NT - 1):
                for i in range(n_overlap):
                    value_mm(t, i)

            # ---- phase 2: drain overlapped rows, then the remaining rows ---
            for i in range(NO):
                if i < n_overlap:
                    o_ps = o_acc[i]
                else:
                    o_ps = psum_s.tile([128, VD], F32, tag="S",
                                       name=f"o_tail{i}")
                    for t in range(NT):
                        nc.tensor.matmul(
                            o_ps[:],
                            a_tiles[t][:, i * 128 : (i + 1) * 128],
                            mv[:, t * VD : (t + 1) * VD],
                            start=(t == 0),
                            stop=(t == NT - 1),
                        )
                o_sb = out_pool.tile([128, VD], F16, tag="osb")
                # DVE stages (and downcasts) the output; each row goes out as
                # 2 DMAs so the last row drains on 2 rings.
                nc.vector.tensor_copy(o_sb[:], o_ps[:])
                for c in range(2):
                    nc.scalar.dma_start(
                        out_d[i * 128 : (i + 1) * 128, c * 256 : (c + 1) * 256],
                        o_sb[:, c * 256 : (c + 1) * 256])

    _split_sync_waits(nc)
    return nc


def make_in_maps(m_key, m_val, q_key, p_m_key, p_q_key):
    in_maps = []
    for b in range(B):
        kt = np.empty((KD, 2 * L), np.float16)
        kt[:, :L] = m_key[b].reshape(KD, L)
        kt[:, L:] = p_m_key[b].reshape(KD, L)
        qt = np.empty((KD, 2 * HW), np.float16)
        qt[:, :HW] = q_key[b].reshape(HW, KD).T
        qt[:, HW:] = p_q_key[b].reshape(HW, KD).T
        mvt = np.ascontiguousarray(
            m_val[b].reshape(VD, L).T.astype(np.float16))
        in_maps.append({"kt": kt, "qt": qt, "mvt": mvt})
    return in_maps


def run(inputs, trace: bool = False, n_overlap: int = 4, n_warm: int = 12):
    """Run on the 8 NeuronCores; returns (full_output, BassKernelResults)."""
    inputs = {k: np.asarray(v, dtype=np.float32) for k, v in inputs.items()}
    key = (n_overlap, n_warm)
    if key not in _nc_cache:
        _nc_cache[key] = build_nc(n_overlap, n_warm)
    nc = _nc_cache[key]
    in_maps = make_in_maps(
        inputs["m_key"], inputs["m_val"], inputs["q_key"],
        inputs["p_m_key"], inputs["p_q_key"],
    )
    res = run_bass_kernel_spmd(nc, in_maps, list(range(NCORES)), trace=trace)
    q_val = inputs["q_val"]
    outs = []
    for b in range(B):
        mat = np.asarray(res.results[b]["out"]).astype(np.float32)
        attn = mat.reshape(VD, H, W)                 # reinterpret, no transpose
        outs.append(np.concatenate([attn, q_val[b]], axis=0))
    return np.stack(outs), res


def kernel(**inputs) -> np.ndarray:
    out, _ = run(inputs, trace=False)
    return out
